# revision 14
# baseline (speedup 1.0000x reference)
"""GAT (2-layer) Trainium2 Bass kernel — 8-core SPMD.

Sharding: dst nodes across 8 cores (12500 each). Per core, dsts are packed
into 98 windows of 128 (one SBUF partition per dst) by a budget-aware
cross-core packer that minimizes per-(window, src-group) slot columns.
Edge rows are fetched with pipelined gpsimd.dma_gather calls from per-layer
node tables (4 src-groups of 25088 rows to fit int16 indices); attention +
weighted segment-sum run as per-partition DVE ops with lrelu/exp on the
scalar engine. Layer-1 table rows are 256B [a_s bf16 x8 | h fp8 x128];
layer-2 rows are 256B [a_s f32 | h bf16 x64]. Pad slots point at a row with
a_s=-300 => weights ~e^-56, no masking needed. Layer-2 node table is built
per-shard and AllGathered. Layer-2 log-softmax defers the Ln to one batched
pass to avoid activation-table thrash.
"""

import os

import numpy as np
import ml_dtypes

import concourse.bacc as bacc
import concourse.bass as bass
import concourse.mybir as mybir
import concourse.tile as tile
from concourse.bass_utils import run_bass_kernel_spmd
from concourse.masks import make_identity

F32 = mybir.dt.float32
BF16 = mybir.dt.bfloat16
FP8 = mybir.dt.float8e4
I16 = mybir.dt.int16
AX = mybir.AxisListType
OP = mybir.AluOpType
ACT = mybir.ActivationFunctionType

N, E = 100000, 1600000
IN, HID, OUT, HEADS = 256, 16, 64, 8
NEG = 0.2
NCORES = 8
NSH = N // NCORES        # 12500
NGRP = 4
GSZ = N // NGRP          # 25000
NP = 25088               # padded rows per group (196*128)
NW = 98                  # windows per core
SH_ROWS = NW * 128       # 12544
PAD1 = GSZ               # group-local pad row, table1 (25000; rows 25000..25087 zero-x)
PAD2 = NSH               # group-local pad row, table2 (12500 in core 2g's shard)
COLS_BUDGET = 160        # slot columns per gather batch
ROW1 = 128               # bf16 elems per table1 row (256B: a_s 8xbf16 | h 128xfp8 | pad)
ROW2 = 128               # bf16 elems per table2 row (256B: a_s f32 | h2 64xbf16 | pad)
A_S_NEG = -300.0


# ---------------------------------------------------------------- host side
def _layout(src, dst):
    core = dst // NSH
    grp = src // GSZ
    cg_all = np.zeros((NCORES, NSH, NGRP), np.int64)
    np.add.at(cg_all, (core, dst % NSH, grp), 1)

    # seed: quantized-profile sort => similar group-profiles share a window
    asgs, cms = [], []
    for k in range(NCORES):
        cg = cg_all[k]
        perm = np.lexsort((cg[:, 3], cg[:, 2] // 3, cg[:, 1] // 4,
                           cg[:, 0] // 5))[::-1]
        asg = np.empty(NSH, np.int64)
        asg[perm] = np.arange(NSH) // 128
        cm = np.zeros((NW, NGRP), np.int64)
        np.maximum.at(cm, asg, cg)
        asgs.append(asg)
        cms.append(cm)
    # refinement: repack each core into the budgets implied by the others
    BIG = 1 << 60
    for _ in range(2):
        for k in range(NCORES):
            others = np.maximum.reduce(
                [cms[j] for j in range(NCORES) if j != k])
            cg = cg_all[k]
            order = np.argsort(-cg.sum(1), kind="stable")
            used = np.zeros(NW, np.int64)
            cm = np.zeros((NW, NGRP), np.int64)
            asg = np.empty(NSH, np.int64)
            eff = others.copy()
            for d in order:
                c = cg[d]
                delta = np.maximum(0, c[None, :] - eff).sum(1)
                delta[used >= 128] = BIG
                cand = np.flatnonzero(delta == delta.min())
                if len(cand) > 1:
                    slack = (eff[cand]
                             - np.minimum(c[None, :], eff[cand])).sum(1)
                    w = cand[np.argmin(slack)]
                else:
                    w = cand[0]
                asg[d] = w
                used[w] += 1
                nm = np.maximum(cm[w], c)
                if (nm != cm[w]).any():
                    cm[w] = nm
                    eff[w] = np.maximum(others[w], cm[w])
            asgs[k], cms[k] = asg, cm
    Lg = np.maximum.reduce(cms)
    Lw = Lg.sum(axis=1)

    sig = np.empty(N, np.int64)
    poss = []
    for k in range(NCORES):
        asg = asgs[k]
        o = np.lexsort((np.arange(NSH), asg))
        counts = np.bincount(asg, minlength=NW)
        starts = np.concatenate([[0], np.cumsum(counts)[:-1]])
        r = np.arange(NSH) - np.repeat(starts, counts)
        pos = np.empty(NSH, np.int64)
        pos[o] = asg[o] * 128 + r
        poss.append(pos)
        sig[k * NSH:(k + 1) * NSH] = k * SH_ROWS + pos

    eorder = np.lexsort((grp, dst))
    es, ed, eg, ec = src[eorder], dst[eorder], grp[eorder], core[eorder]
    core_starts = np.searchsorted(ec, np.arange(NCORES + 1))
    cores = [(es[a:b], (ed[a:b] - k * NSH), eg[a:b])
             for k, (a, b) in enumerate(zip(core_starts[:-1], core_starts[1:]))]
    return dict(Lg=Lg, Lw=Lw, poss=poss, sig=sig, cores=cores)


def _pack_idx(arr_pj):
    """[128, cols] slot-array of indices -> wrapped idx tile [128, cols*8]."""
    I = arr_pj.T.ravel()                      # I[j*128+p]
    W = I.reshape(-1, 16).T.astype(np.int16)  # [16, len/16]
    return np.tile(W, (8, 1))


def _host_inputs(inputs, lay, batches):
    x = np.asarray(inputs["x"], np.float32)
    W1 = np.asarray(inputs["W1"], np.float64)
    att1_s = np.asarray(inputs["att1_s"], np.float64)
    att1_d = np.asarray(inputs["att1_d"], np.float64)
    W2 = np.asarray(inputs["W2"], np.float64)
    att2_s = np.asarray(inputs["att2_s"], np.float64)
    att2_d = np.asarray(inputs["att2_d"], np.float64)
    b1 = np.asarray(inputs["b1"], np.float32)
    b2 = np.asarray(inputs["b2"], np.float32)
    Lg, poss, sig = lay["Lg"], lay["poss"], lay["sig"]

    A_s = np.zeros((HEADS * HID, HEADS))
    A_d = np.zeros((HEADS * HID, HEADS))
    for h in range(HEADS):
        A_s[h * HID:(h + 1) * HID, h] = att1_s[h]
        A_d[h * HID:(h + 1) * HID, h] = att1_d[h]
    w1r = np.concatenate([W1, W1 @ A_s, W1 @ A_d], axis=1)          # [256,144]
    w2r = np.concatenate([W2, W2 @ att2_s.T, W2 @ att2_d.T], axis=1)  # [128,66]
    w1r_bf = w1r.astype(ml_dtypes.bfloat16)
    w2r_bf = w2r.astype(ml_dtypes.bfloat16)

    xT = np.zeros((IN, NGRP * NP), np.float32)
    for g in range(NGRP):
        xT[:, g * NP:g * NP + GSZ] = x[g * GSZ:(g + 1) * GSZ].T
    xT_bf = xT.astype(ml_dtypes.bfloat16)

    common = {
        "xt0": np.ascontiguousarray(xT_bf[:128]),
        "xt1": np.ascontiguousarray(xT_bf[128:]),
        "w1r0": np.ascontiguousarray(w1r_bf[:128]),
        "w1r1": np.ascontiguousarray(w1r_bf[128:]),
        "w2r": np.ascontiguousarray(w2r_bf),
        "b1rep": np.ascontiguousarray(
            np.tile(b1[None, :], (128, 1)).astype(ml_dtypes.bfloat16)),
        "b2rep": np.ascontiguousarray(
            np.tile(b2[None, :], (128, 1)).astype(np.float32)),
    }

    per_core = []
    for k in range(NCORES):
        es, edl, eg = lay["cores"][k]
        pos = poss[k]
        o = np.lexsort((eg, pos[edl]))
        es_o, eg_o, pos_o = es[o], eg[o], pos[edl][o]
        w_o, p_o = pos_o // 128, pos_o % 128
        key = pos_o * NGRP + eg_o
        slot = np.arange(len(o)) - np.searchsorted(key, key)
        idx1_secs, idx2_secs = [], []
        for ws in batches:
            for g in range(NGRP):
                cols = int(Lg[ws, g].sum())
                if cols == 0:
                    continue
                a1 = np.full((128, cols), PAD1, np.int64)
                a2 = np.full((128, cols), PAD2, np.int64)
                coff = 0
                for w in ws:
                    m = (w_o == w) & (eg_o == g)
                    pp, jj, ss = p_o[m], slot[m], es_o[m]
                    a1[pp, coff + jj] = ss % GSZ
                    a2[pp, coff + jj] = sig[ss] % NP
                    coff += int(Lg[w, g])
                idx1_secs.append(a1)
                idx2_secs.append(a2)
        idx1 = np.concatenate([_pack_idx(a) for a in idx1_secs], axis=1)
        idx2 = np.concatenate([_pack_idx(a) for a in idx2_secs], axis=1)
        xtp = np.zeros((IN, SH_ROWS), np.float32)
        xtp[:, pos] = x[k * NSH:(k + 1) * NSH].T
        xtp_bf = xtp.astype(ml_dtypes.bfloat16)
        d = dict(common)
        d["idx1"] = np.ascontiguousarray(idx1)
        d["idx2"] = np.ascontiguousarray(idx2)
        d["xtp0"] = np.ascontiguousarray(xtp_bf[:128])
        d["xtp1"] = np.ascontiguousarray(xtp_bf[128:])
        per_core.append(d)
    return per_core


# ------------------------------------------------------------- device side
def _build_program(Lg, Lw, batches):
    nc = bacc.Bacc("TRN2", target_bir_lowering=False, debug=False,
                   num_devices=NCORES)
    IDXF = int(Lg.sum()) * 8
    LWMAX = int(Lw.max())
    MAXC = max(COLS_BUDGET, LWMAX)
    SIMINIT = int(os.environ.get("GAT_SIMINIT", "0"))
    xt0 = nc.declare_dram_parameter("xt0", [128, NGRP * NP], BF16, isOutput=False)
    xt1 = nc.declare_dram_parameter("xt1", [128, NGRP * NP], BF16, isOutput=False)
    w1r0 = nc.declare_dram_parameter("w1r0", [128, 144], BF16, isOutput=False)
    w1r1 = nc.declare_dram_parameter("w1r1", [128, 144], BF16, isOutput=False)
    w2r = nc.declare_dram_parameter("w2r", [128, 66], BF16, isOutput=False)
    b1rep = nc.declare_dram_parameter("b1rep", [128, 128], BF16, isOutput=False)
    b2rep = nc.declare_dram_parameter("b2rep", [128, 64], F32, isOutput=False)
    idx1 = nc.declare_dram_parameter("idx1", [128, IDXF], I16, isOutput=False)
    idx2 = nc.declare_dram_parameter("idx2", [128, IDXF], I16, isOutput=False)
    xtp0 = nc.declare_dram_parameter("xtp0", [128, SH_ROWS], BF16, isOutput=False)
    xtp1 = nc.declare_dram_parameter("xtp1", [128, SH_ROWS], BF16, isOutput=False)
    outp = nc.declare_dram_parameter("out", [SH_ROWS, OUT], F32, isOutput=True)

    table1 = nc.dram_tensor("table1", [NGRP * NP, ROW1], BF16)
    shard2 = nc.dram_tensor("shard2", [SH_ROWS, ROW2], BF16)
    table2 = nc.dram_tensor("table2", [NCORES * SH_ROWS, ROW2], BF16,
                            addr_space="Shared")

    dma_sems = [nc.alloc_semaphore("g_dmaA"), nc.alloc_semaphore("g_dmaB")]
    prep_sem = nc.alloc_semaphore("g_prep")
    cc_sem = nc.alloc_semaphore("cc")
    gcnt = [0, 0]
    pcnt = [0]
    bpar = [0]

    TPB, BLK = 196, 14

    with tile.TileContext(nc) as tc:
        with (
            tc.tile_pool(name="const", bufs=1) as constp,
            tc.tile_pool(name="psum", bufs=2, space="PSUM") as psump,
            tc.tile_pool(name="stag", bufs=2) as stagp,
            tc.tile_pool(name="idx", bufs=8) as idxp,
            tc.tile_pool(name="work", bufs=2) as workp,
            tc.tile_pool(name="small", bufs=3) as smallp,
        ):
            w1r0_t = constp.tile([128, 144], BF16, tag="w1r0")
            w1r1_t = constp.tile([128, 144], BF16, tag="w1r1")
            w2r_t = constp.tile([128, 66], BF16, tag="w2r")
            b1_t = constp.tile([128, 128], BF16, tag="b1")
            b2_t = constp.tile([128, 64], F32, tag="b2")
            ident = constp.tile([128, 128], BF16, tag="ident")
            adwin = constp.tile([128, NW * HEADS], BF16, tag="adwin")
            ad2win = constp.tile([128, NW], F32, tag="ad2win")
            shbuf = constp.tile([128, NW * OUT], F32, tag="shbuf")
            sebuf = constp.tile([128, NW], F32, tag="sebuf")
            nc.sync.dma_start(out=w1r0_t[:], in_=w1r0[:])
            nc.sync.dma_start(out=w1r1_t[:], in_=w1r1[:])
            nc.sync.dma_start(out=w2r_t[:], in_=w2r[:])
            nc.sync.dma_start(out=b1_t[:], in_=b1rep[:])
            nc.sync.dma_start(out=b2_t[:], in_=b2rep[:])
            make_identity(nc, ident[:])

            # ---------------- phase 0: dense h1 table (all nodes) ----------
            with (
                tc.tile_pool(name="xt", bufs=2) as xtpool,
                tc.tile_pool(name="dense", bufs=2) as densep,
            ):
                for g in range(NGRP):
                    for blk in range(TPB // BLK):
                        base = g * NP + blk * BLK * 128
                        xs0 = xtpool.tile([128, BLK * 128], BF16, tag="xs0")
                        xs1 = xtpool.tile([128, BLK * 128], BF16, tag="xs1")
                        nc.sync.dma_start(out=xs0[:],
                                          in_=xt0[:, base:base + BLK * 128])
                        nc.sync.dma_start(out=xs1[:],
                                          in_=xt1[:, base:base + BLK * 128])
                        rows = densep.tile([128, BLK * ROW1], BF16, tag="rows")
                        for t in range(BLK):
                            ps = psump.tile([128, 144], F32, tag="ps0")
                            nc.tensor.matmul(
                                out=ps[:], lhsT=xs0[:, t * 128:(t + 1) * 128],
                                rhs=w1r0_t[:], start=True, stop=False)
                            nc.tensor.matmul(
                                out=ps[:], lhsT=xs1[:, t * 128:(t + 1) * 128],
                                rhs=w1r1_t[:], start=False, stop=True)
                            rv = rows[:, t * ROW1:(t + 1) * ROW1]
                            nc.scalar.activation(rv[:, 0:8], ps[:, 128:136],
                                                 ACT.Copy)
                            nc.scalar.activation(rv[:, 8:72].bitcast(FP8),
                                                 ps[:, 0:128], ACT.Copy)
                            if SIMINIT:
                                nc.vector.memset(rv[:, 72:128], 0.0)
                        nc.sync.dma_start(
                            out=table1[base:base + BLK * 128, :]
                                .rearrange("(a p) r -> p a r", p=128),
                            in_=rows[:].rearrange("p (a r) -> p a r", a=BLK))
                # pad row: a_s := -300 (h stays 0) on group-local row PAD1
                padrow = constp.tile([128, ROW1], BF16, tag="padrow")
                nc.vector.memset(padrow[:], 0.0)
                nc.vector.memset(padrow[0:1, 0:8], A_S_NEG)
                for g in range(NGRP):
                    nc.sync.dma_start(
                        out=table1[g * NP + PAD1:g * NP + PAD1 + 1, :],
                        in_=padrow[0:1, :])

                # a_d per window (window-ordered x.T)
                for w in range(NW):
                    xp0 = xtpool.tile([128, 128], BF16, tag="xp0")
                    xp1 = xtpool.tile([128, 128], BF16, tag="xp1")
                    nc.sync.dma_start(out=xp0[:],
                                      in_=xtp0[:, w * 128:(w + 1) * 128])
                    nc.sync.dma_start(out=xp1[:],
                                      in_=xtp1[:, w * 128:(w + 1) * 128])
                    psa = psump.tile([128, 16], F32, tag="psa")
                    nc.tensor.matmul(out=psa[:], lhsT=xp0[:],
                                     rhs=w1r0_t[:, 128:144],
                                     start=True, stop=False)
                    nc.tensor.matmul(out=psa[:], lhsT=xp1[:],
                                     rhs=w1r1_t[:, 128:144],
                                     start=False, stop=True)
                    nc.scalar.activation(adwin[:, w * 8:(w + 1) * 8],
                                         psa[:, 8:16], ACT.Copy)

            # ---------------- edge layers ----------------------------------
            def edge_layer(layer):
                tabl = table1 if layer == 1 else table2
                idxin = idx1 if layer == 1 else idx2
                nh = HEADS if layer == 1 else 1
                nch = HID if layer == 1 else OUT
                idx_off = 0

                def compute(pd):
                    ws, gbase, stag = pd["ws"], pd["gbase"], pd["stag"]
                    woff = np.zeros(NGRP, np.int64)
                    for w in ws:
                        Lwv = int(Lw[w])
                        wall_t = workp.tile([128, LWMAX * HEADS], BF16, tag="wa")
                        wall = wall_t[:, 0:Lwv * nh]
                        msg_t = workp.tile([128, LWMAX * HEADS * HID], BF16,
                                           tag="mg")
                        msg = msg_t[:, 0:Lwv * nh * nch]
                        if Lwv > 0:
                            wsec = 0
                            for g in range(NGRP):
                                Lgv = int(Lg[w, g])
                                if Lgv == 0:
                                    continue
                                c0 = int(gbase[g] + woff[g])
                                sl3 = stag[:, c0 * ROW1:(c0 + Lgv) * ROW1] \
                                    .rearrange("p (l r) -> p l r", l=Lgv)
                                if layer == 1:
                                    a_s = sl3[:, :, 0:8]
                                    adv = adwin[:, w * 8:(w + 1) * 8]
                                else:
                                    a_s = sl3[:, :, 0:2].bitcast(F32)
                                    adv = ad2win[:, w:w + 1]
                                adv = adv.rearrange("p (l h) -> p l h", l=1) \
                                    .to_broadcast([128, Lgv, nh])
                                uv = wall_t[:, wsec * nh:(wsec + Lgv) * nh] \
                                    .rearrange("p (l h) -> p l h", l=Lgv)
                                nc.vector.tensor_tensor(out=uv, in0=a_s,
                                                        in1=adv, op=OP.add)
                                wsec += Lgv
                            lr = workp.tile([128, LWMAX * HEADS], BF16,
                                            tag="lr")
                            nc.vector.tensor_scalar_mul(lr[:, 0:Lwv * nh],
                                                        wall, NEG)
                            nc.vector.tensor_tensor(out=wall, in0=wall,
                                                    in1=lr[:, 0:Lwv * nh],
                                                    op=OP.max)
                            if layer == 2:
                                den = smallp.tile([128, HEADS], F32, tag="den")
                                nc.scalar.activation(
                                    wall, wall, ACT.Exp, 0.0, 1.0,
                                    accum_out=den[:, 0:1])
                            else:
                                nc.scalar.activation(wall, wall, ACT.Exp,
                                                     0.0, 1.0)
                            # weighted messages
                            wsec = 0
                            for g in range(NGRP):
                                Lgv = int(Lg[w, g])
                                if Lgv == 0:
                                    continue
                                c0 = int(gbase[g] + woff[g])
                                sl3 = stag[:, c0 * ROW1:(c0 + Lgv) * ROW1] \
                                    .rearrange("p (l r) -> p l r", l=Lgv)
                                if layer == 1:
                                    hv = sl3[:, :, 8:72].bitcast(FP8) \
                                        .rearrange("p l (h c) -> p l h c", h=nh)
                                else:
                                    hv = sl3[:, :, 2:66] \
                                        .rearrange("p l (h c) -> p l h c", h=nh)
                                wv = wall_t[:, wsec * nh:(wsec + Lgv) * nh] \
                                    .rearrange("p (l h c) -> p l h c",
                                               l=Lgv, h=nh, c=1) \
                                    .to_broadcast([128, Lgv, nh, nch])
                                mv = msg_t[:, wsec * nh * nch:
                                           (wsec + Lgv) * nh * nch] \
                                    .rearrange("p (l h c) -> p l h c",
                                               l=Lgv, h=nh)
                                nc.vector.tensor_tensor(out=mv, in0=hv,
                                                        in1=wv, op=OP.mult)
                                wsec += Lgv
                                woff[g] += Lgv
                            if layer == 1:
                                den = smallp.tile([128, HEADS], F32, tag="den")
                                nc.vector.tensor_reduce(
                                    out=den[:, 0:nh],
                                    in_=wall.rearrange("p (l h) -> p h l",
                                                       l=Lwv),
                                    axis=AX.X, op=OP.add)
                            opre = smallp.tile([128, HEADS * HID], F32,
                                               tag="opre")
                            nc.vector.tensor_reduce(
                                out=opre[:, 0:nh * nch],
                                in_=msg.rearrange("p (l h c) -> p h c l",
                                                  l=Lwv, h=nh),
                                axis=AX.X, op=OP.add)
                        else:
                            den = smallp.tile([128, HEADS], F32, tag="den")
                            opre = smallp.tile([128, HEADS * HID], F32,
                                               tag="opre")
                            nc.vector.memset(den[:, 0:nh], 0.0)
                            nc.vector.memset(opre[:, 0:nh * nch], 0.0)
                        nc.vector.tensor_scalar_max(den[:, 0:nh],
                                                    den[:, 0:nh], 1e-30)
                        rec = smallp.tile([128, HEADS], F32, tag="rec")
                        nc.vector.reciprocal(rec[:, 0:nh], den[:, 0:nh])
                        if layer == 1:
                            o1 = smallp.tile([128, 128], BF16, tag="o1")
                            nc.vector.tensor_tensor(
                                out=o1[:].rearrange("p (h c) -> p h c", h=nh),
                                in0=opre[:].rearrange("p (h c) -> p h c",
                                                      h=nh),
                                in1=rec[:].rearrange("p (h c) -> p h c", c=1)
                                    .to_broadcast([128, nh, nch]),
                                op=OP.mult)
                            nc.vector.tensor_tensor(out=o1[:], in0=o1[:],
                                                    in1=b1_t[:], op=OP.add)
                            tneg = smallp.tile([128, 128], BF16, tag="tneg")
                            nc.vector.tensor_scalar_min(tneg[:], o1[:], 0.0)
                            nc.scalar.activation(tneg[:], tneg[:], ACT.Exp,
                                                 0.0, 1.0)
                            nc.vector.tensor_relu(o1[:], o1[:])
                            nc.vector.tensor_tensor(out=o1[:], in0=o1[:],
                                                    in1=tneg[:], op=OP.add)
                            nc.vector.tensor_scalar_add(o1[:], o1[:], -1.0)
                            pst = psump.tile([128, 128], BF16, tag="pst")
                            nc.tensor.transpose(out=pst[:], in_=o1[:],
                                                identity=ident[:])
                            o1T = smallp.tile([128, 128], BF16, tag="o1T")
                            nc.scalar.activation(o1T[:], pst[:], ACT.Copy)
                            ps2 = psump.tile([128, 66], F32, tag="ps2")
                            nc.tensor.matmul(out=ps2[:], lhsT=o1T[:],
                                             rhs=w2r_t[:],
                                             start=True, stop=True)
                            row2 = smallp.tile([128, ROW2], BF16, tag="row2")
                            nc.scalar.activation(row2[:, 0:2].bitcast(F32),
                                                 ps2[:, 64:65], ACT.Copy)
                            nc.scalar.activation(row2[:, 2:66],
                                                 ps2[:, 0:64], ACT.Copy)
                            if SIMINIT:
                                nc.vector.memset(row2[:, 66:128], 0.0)
                            nc.vector.tensor_copy(out=ad2win[:, w:w + 1],
                                                  in_=ps2[:, 65:66])
                            nc.sync.dma_start(
                                out=shard2[w * 128:(w + 1) * 128, :],
                                in_=row2[:])
                        else:
                            o1v = shbuf[:, w * OUT:(w + 1) * OUT]
                            nc.vector.tensor_scalar(
                                out=o1v, in0=opre[:, 0:OUT],
                                scalar1=rec[:, 0:1], scalar2=None,
                                op0=OP.mult)
                            nc.vector.tensor_tensor(out=o1v, in0=o1v,
                                                    in1=b2_t[:], op=OP.add)
                            ex = smallp.tile([128, OUT], F32, tag="ex")
                            nc.scalar.activation(
                                ex[:], o1v, ACT.Exp, 0.0, 1.0,
                                accum_out=sebuf[:, w:w + 1])

                pend = None
                for ws in batches:
                    par = bpar[0] % 2
                    bpar[0] += 1
                    gbase = np.concatenate(
                        [[0], np.cumsum([int(Lg[ws, g].sum())
                                         for g in range(NGRP)])])
                    stag = stagp.tile([128, MAXC * ROW1], BF16, tag="st")
                    gaths = []
                    for g in range(NGRP):
                        cols = int(Lg[ws, g].sum())
                        if cols == 0:
                            continue
                        nidx = 128 * cols
                        ixt = idxp.tile([128, MAXC * 8], I16, tag="ix")
                        nc.sync.dma_start(
                            out=ixt[:, 0:nidx // 16],
                            in_=idxin[:, idx_off:idx_off + nidx // 16])
                        idx_off += nidx // 16
                        sl = stag[:, int(gbase[g]) * ROW1:
                                  (int(gbase[g]) + cols) * ROW1]
                        sl3 = sl.rearrange("p (k d) -> p k d", d=ROW1)
                        gaths.append((g, sl3, ixt, nidx))
                    with tc.tile_critical():
                        for g, sl3, ixt, nidx in gaths:
                            nc.gpsimd.dma_gather(
                                out_ap=sl3, in_ap=tabl[g * NP:(g + 1) * NP, :],
                                idxs_ap=ixt[:, 0:nidx // 16],
                                num_idxs=nidx, num_idxs_reg=nidx,
                                elem_size=ROW1, single_packet=False,
                                prepare_only=True, sem=dma_sems[par],
                            ).then_inc(prep_sem, 1)
                            pcnt[0] += 1
                            nc.gpsimd.wait_ge(prep_sem, pcnt[0])
                            nc.gpsimd.trigger_dma(count=1)
                            gcnt[par] += 1
                        if pend is not None:
                            nc.gpsimd.wait_ge(dma_sems[pend["par"]],
                                              16 * pend["gc"])
                            for sl3p in pend["slices"]:
                                nc.gpsimd.tensor_copy(out=sl3p[:, :, 0:1],
                                                      in_=sl3p[:, :, 0:1])
                            for ixtp in pend["ixts"]:
                                nc.gpsimd.tensor_copy(out=ixtp[0:1, 0:1],
                                                      in_=ixtp[0:1, 0:1])
                    if pend is not None:
                        compute(pend)
                    pend = dict(ws=ws, gbase=gbase, stag=stag,
                                slices=[t[1] for t in gaths],
                                ixts=[t[2] for t in gaths],
                                gc=gcnt[par], par=par)
                with tc.tile_critical():
                    nc.gpsimd.wait_ge(dma_sems[pend["par"]], 16 * pend["gc"])
                    for sl3p in pend["slices"]:
                        nc.gpsimd.tensor_copy(out=sl3p[:, :, 0:1],
                                              in_=sl3p[:, :, 0:1])
                    for ixtp in pend["ixts"]:
                        nc.gpsimd.tensor_copy(out=ixtp[0:1, 0:1],
                                              in_=ixtp[0:1, 0:1])
                compute(pend)

            edge_layer(1)

            # table2 pad row: a_s := -300
            pr2 = constp.tile([1, 2], BF16, tag="pr2")
            nc.vector.memset(pr2[0:1, 0:2].bitcast(F32), A_S_NEG)
            nc.sync.dma_start(out=shard2[PAD2:PAD2 + 1, 0:2], in_=pr2[0:1, :])

            with tc.tile_critical():
                nc.gpsimd.collective_compute(
                    "AllGather", OP.bypass,
                    replica_groups=[list(range(NCORES))],
                    ins=[shard2[:]], outs=[table2[:]],
                ).then_inc(cc_sem, 1)
                nc.gpsimd.wait_ge(cc_sem, 1)

            edge_layer(2)

            # deferred log-softmax normalizer + output
            lnse = constp.tile([128, NW], F32, tag="lnse")
            nc.scalar.activation(lnse[:], sebuf[:], ACT.Ln, 0.0, 1.0)
            for w in range(NW):
                fo = smallp.tile([128, OUT], F32, tag="fo")
                nc.vector.tensor_scalar(
                    out=fo[:], in0=shbuf[:, w * OUT:(w + 1) * OUT],
                    scalar1=lnse[:, w:w + 1], scalar2=None, op0=OP.subtract)
                nc.sync.dma_start(out=outp[w * 128:(w + 1) * 128, :],
                                  in_=fo[:])
    nc.compile()
    return nc


_CACHE = {}


def _batches(Lw):
    batches = []
    cur, acc = [], 0
    for w in range(NW):
        lw = int(Lw[w])
        if cur and (acc + lw > COLS_BUDGET or len(cur) >= 16):
            batches.append(cur)
            cur, acc = [], 0
        cur.append(w)
        acc += lw
    if cur:
        batches.append(cur)
    return batches


def kernel(**inputs):
    ei = np.asarray(inputs["edge_index"])
    src, dst = ei[0].astype(np.int64), ei[1].astype(np.int64)
    lay = _layout(src, dst)
    batches = _batches(lay["Lw"])
    per_core = _host_inputs(inputs, lay, batches)
    key = (ei.tobytes()[:64], int(lay["Lg"].sum()))
    if key not in _CACHE:
        _CACHE[key] = _build_program(lay["Lg"], lay["Lw"], batches)
    nc = _CACHE[key]
    res = run_bass_kernel_spmd(nc, per_core, core_ids=list(range(NCORES)))
    out = np.empty((N, OUT), np.float32)
    for k in range(NCORES):
        out[k * NSH:(k + 1) * NSH] = res.results[k]["out"][lay["poss"][k]]
    return out


if __name__ == "__main__":
    d = np.load("/root/problem/inputs.npz")
    o = kernel(**{k: d[k] for k in d.files})
    ref = np.load("/root/problem/ref_out.npy")
    rel = np.linalg.norm(o - ref) / np.linalg.norm(ref)
    err = np.abs(o - ref) / (np.abs(ref) + 1e-5)
    print("fro rel err:", rel, "max elem rel err:", err.max())


# revision 15
# speedup vs baseline: 1.0126x; 1.0126x over previous
"""GAT (2-layer) Trainium2 Bass kernel — 8-core SPMD.

Sharding: dst nodes across 8 cores (12500 each). Per core, dsts are packed
into 98 windows of 128 (one SBUF partition per dst) by a budget-aware
cross-core packer that minimizes per-(window, src-group) slot columns.
Edge rows are fetched with pipelined gpsimd.dma_gather calls from per-layer
node tables (4 src-groups of 25088 rows to fit int16 indices); attention +
weighted segment-sum run as per-partition DVE ops with lrelu/exp on the
scalar engine. Layer-1 table rows are 256B [a_s bf16 x8 | h fp8 x128];
layer-2 rows are 256B [a_s f32 | h bf16 x64]. Pad slots point at a row with
a_s=-300 => weights ~e^-56, no masking needed. Layer-2 node table is built
per-shard and AllGathered. Layer-2 log-softmax defers the Ln to one batched
pass to avoid activation-table thrash.
"""

import os

import numpy as np
import ml_dtypes

import concourse.bacc as bacc
import concourse.bass as bass
import concourse.mybir as mybir
import concourse.tile as tile
from concourse.bass_utils import run_bass_kernel_spmd
from concourse.masks import make_identity

F32 = mybir.dt.float32
BF16 = mybir.dt.bfloat16
FP8 = mybir.dt.float8e4
I16 = mybir.dt.int16
AX = mybir.AxisListType
OP = mybir.AluOpType
ACT = mybir.ActivationFunctionType

N, E = 100000, 1600000
IN, HID, OUT, HEADS = 256, 16, 64, 8
NEG = 0.2
NCORES = 8
NSH = N // NCORES        # 12500
NGRP = 4
GSZ = N // NGRP          # 25000
NP = 25088               # padded rows per group (196*128)
NW = 98                  # windows per core
SH_ROWS = NW * 128       # 12544
PAD1 = GSZ               # group-local pad row, table1 (25000; rows 25000..25087 zero-x)
PAD2 = NSH               # group-local pad row, table2 (12500 in core 2g's shard)
COLS_BUDGET = 160        # slot columns per gather batch
ROW1 = 128               # bf16 elems per table1 row (256B: a_s 8xbf16 | h 128xfp8 | pad)
ROW2 = 128               # bf16 elems per table2 row (256B: a_s f32 | h2 64xbf16 | pad)
A_S_NEG = -300.0


# ---------------------------------------------------------------- host side
def _layout(src, dst):
    core = dst // NSH
    grp = src // GSZ
    cg_all = np.zeros((NCORES, NSH, NGRP), np.int64)
    np.add.at(cg_all, (core, dst % NSH, grp), 1)

    # seed: quantized-profile sort => similar group-profiles share a window
    asgs, cms = [], []
    for k in range(NCORES):
        cg = cg_all[k]
        perm = np.lexsort((cg[:, 3], cg[:, 2] // 3, cg[:, 1] // 4,
                           cg[:, 0] // 5))[::-1]
        asg = np.empty(NSH, np.int64)
        asg[perm] = np.arange(NSH) // 128
        cm = np.zeros((NW, NGRP), np.int64)
        np.maximum.at(cm, asg, cg)
        asgs.append(asg)
        cms.append(cm)
    # refinement: repack each core into the budgets implied by the others
    BIG = 1 << 60
    for _ in range(2):
        for k in range(NCORES):
            others = np.maximum.reduce(
                [cms[j] for j in range(NCORES) if j != k])
            cg = cg_all[k]
            order = np.argsort(-cg.sum(1), kind="stable")
            used = np.zeros(NW, np.int64)
            cm = np.zeros((NW, NGRP), np.int64)
            asg = np.empty(NSH, np.int64)
            eff = others.copy()
            for d in order:
                c = cg[d]
                delta = np.maximum(0, c[None, :] - eff).sum(1)
                delta[used >= 128] = BIG
                cand = np.flatnonzero(delta == delta.min())
                if len(cand) > 1:
                    slack = (eff[cand]
                             - np.minimum(c[None, :], eff[cand])).sum(1)
                    w = cand[np.argmin(slack)]
                else:
                    w = cand[0]
                asg[d] = w
                used[w] += 1
                nm = np.maximum(cm[w], c)
                if (nm != cm[w]).any():
                    cm[w] = nm
                    eff[w] = np.maximum(others[w], cm[w])
            asgs[k], cms[k] = asg, cm
    Lg = np.maximum.reduce(cms)
    Lw = Lg.sum(axis=1)

    sig = np.empty(N, np.int64)
    poss = []
    for k in range(NCORES):
        asg = asgs[k]
        o = np.lexsort((np.arange(NSH), asg))
        counts = np.bincount(asg, minlength=NW)
        starts = np.concatenate([[0], np.cumsum(counts)[:-1]])
        r = np.arange(NSH) - np.repeat(starts, counts)
        pos = np.empty(NSH, np.int64)
        pos[o] = asg[o] * 128 + r
        poss.append(pos)
        sig[k * NSH:(k + 1) * NSH] = k * SH_ROWS + pos

    eorder = np.lexsort((grp, dst))
    es, ed, eg, ec = src[eorder], dst[eorder], grp[eorder], core[eorder]
    core_starts = np.searchsorted(ec, np.arange(NCORES + 1))
    cores = [(es[a:b], (ed[a:b] - k * NSH), eg[a:b])
             for k, (a, b) in enumerate(zip(core_starts[:-1], core_starts[1:]))]
    return dict(Lg=Lg, Lw=Lw, poss=poss, sig=sig, cores=cores)


def _pack_idx(arr_pj):
    """[128, cols] slot-array of indices -> wrapped idx tile [128, cols*8]."""
    I = arr_pj.T.ravel()                      # I[j*128+p]
    W = I.reshape(-1, 16).T.astype(np.int16)  # [16, len/16]
    return np.tile(W, (8, 1))


def _host_inputs(inputs, lay, batches):
    x = np.asarray(inputs["x"], np.float32)
    W1 = np.asarray(inputs["W1"], np.float64)
    att1_s = np.asarray(inputs["att1_s"], np.float64)
    att1_d = np.asarray(inputs["att1_d"], np.float64)
    W2 = np.asarray(inputs["W2"], np.float64)
    att2_s = np.asarray(inputs["att2_s"], np.float64)
    att2_d = np.asarray(inputs["att2_d"], np.float64)
    b1 = np.asarray(inputs["b1"], np.float32)
    b2 = np.asarray(inputs["b2"], np.float32)
    Lg, poss, sig = lay["Lg"], lay["poss"], lay["sig"]

    A_s = np.zeros((HEADS * HID, HEADS))
    A_d = np.zeros((HEADS * HID, HEADS))
    for h in range(HEADS):
        A_s[h * HID:(h + 1) * HID, h] = att1_s[h]
        A_d[h * HID:(h + 1) * HID, h] = att1_d[h]
    w1r = np.concatenate([W1, W1 @ A_s, W1 @ A_d], axis=1)          # [256,144]
    w2r = np.concatenate([W2, W2 @ att2_s.T, W2 @ att2_d.T], axis=1)  # [128,66]
    w1r_bf = w1r.astype(ml_dtypes.bfloat16)
    w2r_bf = w2r.astype(ml_dtypes.bfloat16)

    xT = np.zeros((IN, NGRP * NP), np.float32)
    for g in range(NGRP):
        xT[:, g * NP:g * NP + GSZ] = x[g * GSZ:(g + 1) * GSZ].T
    xT_bf = xT.astype(ml_dtypes.bfloat16)

    common = {
        "xt0": np.ascontiguousarray(xT_bf[:128]),
        "xt1": np.ascontiguousarray(xT_bf[128:]),
        "w1r0": np.ascontiguousarray(w1r_bf[:128]),
        "w1r1": np.ascontiguousarray(w1r_bf[128:]),
        "w2r": np.ascontiguousarray(w2r_bf),
        "b1rep": np.ascontiguousarray(
            np.tile(b1[None, :], (128, 1)).astype(ml_dtypes.bfloat16)),
        "b2rep": np.ascontiguousarray(
            np.tile(b2[None, :], (128, 1)).astype(np.float32)),
    }

    per_core = []
    for k in range(NCORES):
        es, edl, eg = lay["cores"][k]
        pos = poss[k]
        o = np.lexsort((eg, pos[edl]))
        es_o, eg_o, pos_o = es[o], eg[o], pos[edl][o]
        w_o, p_o = pos_o // 128, pos_o % 128
        key = pos_o * NGRP + eg_o
        slot = np.arange(len(o)) - np.searchsorted(key, key)
        idx1_secs, idx2_secs = [], []
        for ws in batches:
            for g in range(NGRP):
                cols = int(Lg[ws, g].sum())
                if cols == 0:
                    continue
                a1 = np.full((128, cols), PAD1, np.int64)
                a2 = np.full((128, cols), PAD2, np.int64)
                coff = 0
                for w in ws:
                    m = (w_o == w) & (eg_o == g)
                    pp, jj, ss = p_o[m], slot[m], es_o[m]
                    a1[pp, coff + jj] = ss % GSZ
                    a2[pp, coff + jj] = sig[ss] % NP
                    coff += int(Lg[w, g])
                idx1_secs.append(a1)
                idx2_secs.append(a2)
        idx1 = np.concatenate([_pack_idx(a) for a in idx1_secs], axis=1)
        idx2 = np.concatenate([_pack_idx(a) for a in idx2_secs], axis=1)
        xtp = np.zeros((IN, SH_ROWS), np.float32)
        xtp[:, pos] = x[k * NSH:(k + 1) * NSH].T
        xtp_bf = xtp.astype(ml_dtypes.bfloat16)
        d = dict(common)
        d["idx1"] = np.ascontiguousarray(idx1)
        d["idx2"] = np.ascontiguousarray(idx2)
        d["xtp0"] = np.ascontiguousarray(xtp_bf[:128])
        d["xtp1"] = np.ascontiguousarray(xtp_bf[128:])
        per_core.append(d)
    return per_core


# ------------------------------------------------------------- device side
def _build_program(Lg, Lw, batches):
    nc = bacc.Bacc("TRN2", target_bir_lowering=False, debug=False,
                   num_devices=NCORES)
    IDXF = int(Lg.sum()) * 8
    LWMAX = int(Lw.max())
    MAXC = max(COLS_BUDGET, LWMAX)
    SIMINIT = int(os.environ.get("GAT_SIMINIT", "0"))
    xt0 = nc.declare_dram_parameter("xt0", [128, NGRP * NP], BF16, isOutput=False)
    xt1 = nc.declare_dram_parameter("xt1", [128, NGRP * NP], BF16, isOutput=False)
    w1r0 = nc.declare_dram_parameter("w1r0", [128, 144], BF16, isOutput=False)
    w1r1 = nc.declare_dram_parameter("w1r1", [128, 144], BF16, isOutput=False)
    w2r = nc.declare_dram_parameter("w2r", [128, 66], BF16, isOutput=False)
    b1rep = nc.declare_dram_parameter("b1rep", [128, 128], BF16, isOutput=False)
    b2rep = nc.declare_dram_parameter("b2rep", [128, 64], F32, isOutput=False)
    idx1 = nc.declare_dram_parameter("idx1", [128, IDXF], I16, isOutput=False)
    idx2 = nc.declare_dram_parameter("idx2", [128, IDXF], I16, isOutput=False)
    xtp0 = nc.declare_dram_parameter("xtp0", [128, SH_ROWS], BF16, isOutput=False)
    xtp1 = nc.declare_dram_parameter("xtp1", [128, SH_ROWS], BF16, isOutput=False)
    outp = nc.declare_dram_parameter("out", [SH_ROWS, OUT], F32, isOutput=True)

    table1 = nc.dram_tensor("table1", [NGRP * NP, ROW1], BF16)
    shard2 = nc.dram_tensor("shard2", [SH_ROWS, ROW2], BF16)
    table2 = nc.dram_tensor("table2", [NCORES * SH_ROWS, ROW2], BF16,
                            addr_space="Shared")

    dma_sems = [nc.alloc_semaphore("g_dmaA"), nc.alloc_semaphore("g_dmaB")]
    prep_sem = nc.alloc_semaphore("g_prep")
    cc_sem = nc.alloc_semaphore("cc")
    gcnt = [0, 0]
    pcnt = [0]
    bpar = [0]

    TPB, BLK = 196, 14

    with tile.TileContext(nc) as tc:
        with (
            tc.tile_pool(name="const", bufs=1) as constp,
            tc.tile_pool(name="psum", bufs=2, space="PSUM") as psump,
            tc.tile_pool(name="stag", bufs=2) as stagp,
            tc.tile_pool(name="idx", bufs=8) as idxp,
            tc.tile_pool(name="work", bufs=2) as workp,
            tc.tile_pool(name="small", bufs=3) as smallp,
        ):
            w1r0_t = constp.tile([128, 144], BF16, tag="w1r0")
            w1r1_t = constp.tile([128, 144], BF16, tag="w1r1")
            w2r_t = constp.tile([128, 66], BF16, tag="w2r")
            b1_t = constp.tile([128, 128], BF16, tag="b1")
            b2_t = constp.tile([128, 64], F32, tag="b2")
            ident = constp.tile([128, 128], BF16, tag="ident")
            adwin = constp.tile([128, NW * HEADS], BF16, tag="adwin")
            ad2win = constp.tile([128, NW], F32, tag="ad2win")
            shbuf = constp.tile([128, NW * OUT], F32, tag="shbuf")
            sebuf = constp.tile([128, NW], F32, tag="sebuf")
            nc.sync.dma_start(out=w1r0_t[:], in_=w1r0[:])
            nc.sync.dma_start(out=w1r1_t[:], in_=w1r1[:])
            nc.sync.dma_start(out=w2r_t[:], in_=w2r[:])
            nc.sync.dma_start(out=b1_t[:], in_=b1rep[:])
            nc.sync.dma_start(out=b2_t[:], in_=b2rep[:])
            make_identity(nc, ident[:])

            # ---------------- phase 0: dense h1 table (all nodes) ----------
            with (
                tc.tile_pool(name="xt", bufs=2) as xtpool,
                tc.tile_pool(name="dense", bufs=2) as densep,
            ):
                for g in range(NGRP):
                    for blk in range(TPB // BLK):
                        base = g * NP + blk * BLK * 128
                        xs0 = xtpool.tile([128, BLK * 128], BF16, tag="xs0")
                        xs1 = xtpool.tile([128, BLK * 128], BF16, tag="xs1")
                        nc.sync.dma_start(out=xs0[:],
                                          in_=xt0[:, base:base + BLK * 128])
                        nc.sync.dma_start(out=xs1[:],
                                          in_=xt1[:, base:base + BLK * 128])
                        rows = densep.tile([128, BLK * ROW1], BF16, tag="rows")
                        for t in range(BLK):
                            ps = psump.tile([128, 144], F32, tag="ps0")
                            nc.tensor.matmul(
                                out=ps[:], lhsT=xs0[:, t * 128:(t + 1) * 128],
                                rhs=w1r0_t[:], start=True, stop=False)
                            nc.tensor.matmul(
                                out=ps[:], lhsT=xs1[:, t * 128:(t + 1) * 128],
                                rhs=w1r1_t[:], start=False, stop=True)
                            rv = rows[:, t * ROW1:(t + 1) * ROW1]
                            if t % 2 == 0:
                                nc.scalar.activation(rv[:, 0:8],
                                                     ps[:, 128:136], ACT.Copy)
                                nc.scalar.activation(rv[:, 8:72].bitcast(FP8),
                                                     ps[:, 0:128], ACT.Copy)
                            else:
                                nc.vector.tensor_copy(out=rv[:, 0:8],
                                                      in_=ps[:, 128:136])
                                nc.vector.tensor_copy(
                                    out=rv[:, 8:72].bitcast(FP8),
                                    in_=ps[:, 0:128])
                            if SIMINIT:
                                nc.vector.memset(rv[:, 72:128], 0.0)
                        nc.sync.dma_start(
                            out=table1[base:base + BLK * 128, :]
                                .rearrange("(a p) r -> p a r", p=128),
                            in_=rows[:].rearrange("p (a r) -> p a r", a=BLK))
                # pad row: a_s := -300 (h stays 0) on group-local row PAD1
                padrow = constp.tile([128, ROW1], BF16, tag="padrow")
                nc.vector.memset(padrow[:], 0.0)
                nc.vector.memset(padrow[0:1, 0:8], A_S_NEG)
                for g in range(NGRP):
                    nc.sync.dma_start(
                        out=table1[g * NP + PAD1:g * NP + PAD1 + 1, :],
                        in_=padrow[0:1, :])

                # a_d per window (window-ordered x.T)
                for w in range(NW):
                    xp0 = xtpool.tile([128, 128], BF16, tag="xp0")
                    xp1 = xtpool.tile([128, 128], BF16, tag="xp1")
                    nc.sync.dma_start(out=xp0[:],
                                      in_=xtp0[:, w * 128:(w + 1) * 128])
                    nc.sync.dma_start(out=xp1[:],
                                      in_=xtp1[:, w * 128:(w + 1) * 128])
                    psa = psump.tile([128, 16], F32, tag="psa")
                    nc.tensor.matmul(out=psa[:], lhsT=xp0[:],
                                     rhs=w1r0_t[:, 128:144],
                                     start=True, stop=False)
                    nc.tensor.matmul(out=psa[:], lhsT=xp1[:],
                                     rhs=w1r1_t[:, 128:144],
                                     start=False, stop=True)
                    nc.scalar.activation(adwin[:, w * 8:(w + 1) * 8],
                                         psa[:, 8:16], ACT.Copy)

            # ---------------- edge layers ----------------------------------
            def edge_layer(layer):
                tabl = table1 if layer == 1 else table2
                idxin = idx1 if layer == 1 else idx2
                nh = HEADS if layer == 1 else 1
                nch = HID if layer == 1 else OUT
                idx_off = 0

                def compute(pd):
                    ws, gbase, stag = pd["ws"], pd["gbase"], pd["stag"]
                    woff = np.zeros(NGRP, np.int64)
                    for w in ws:
                        Lwv = int(Lw[w])
                        wall_t = workp.tile([128, LWMAX * HEADS], BF16, tag="wa")
                        wall = wall_t[:, 0:Lwv * nh]
                        msg_t = workp.tile([128, LWMAX * HEADS * HID], BF16,
                                           tag="mg")
                        msg = msg_t[:, 0:Lwv * nh * nch]
                        if Lwv > 0:
                            wsec = 0
                            for g in range(NGRP):
                                Lgv = int(Lg[w, g])
                                if Lgv == 0:
                                    continue
                                c0 = int(gbase[g] + woff[g])
                                sl3 = stag[:, c0 * ROW1:(c0 + Lgv) * ROW1] \
                                    .rearrange("p (l r) -> p l r", l=Lgv)
                                if layer == 1:
                                    a_s = sl3[:, :, 0:8]
                                    adv = adwin[:, w * 8:(w + 1) * 8]
                                else:
                                    a_s = sl3[:, :, 0:2].bitcast(F32)
                                    adv = ad2win[:, w:w + 1]
                                adv = adv.rearrange("p (l h) -> p l h", l=1) \
                                    .to_broadcast([128, Lgv, nh])
                                uv = wall_t[:, wsec * nh:(wsec + Lgv) * nh] \
                                    .rearrange("p (l h) -> p l h", l=Lgv)
                                nc.vector.tensor_tensor(out=uv, in0=a_s,
                                                        in1=adv, op=OP.add)
                                wsec += Lgv
                            lr = workp.tile([128, LWMAX * HEADS], BF16,
                                            tag="lr")
                            nc.vector.tensor_scalar_mul(lr[:, 0:Lwv * nh],
                                                        wall, NEG)
                            nc.vector.tensor_tensor(out=wall, in0=wall,
                                                    in1=lr[:, 0:Lwv * nh],
                                                    op=OP.max)
                            if layer == 2:
                                den = smallp.tile([128, HEADS], F32, tag="den")
                                nc.scalar.activation(
                                    wall, wall, ACT.Exp, 0.0, 1.0,
                                    accum_out=den[:, 0:1])
                            else:
                                nc.scalar.activation(wall, wall, ACT.Exp,
                                                     0.0, 1.0)
                            # weighted messages
                            wsec = 0
                            for g in range(NGRP):
                                Lgv = int(Lg[w, g])
                                if Lgv == 0:
                                    continue
                                c0 = int(gbase[g] + woff[g])
                                sl3 = stag[:, c0 * ROW1:(c0 + Lgv) * ROW1] \
                                    .rearrange("p (l r) -> p l r", l=Lgv)
                                if layer == 1:
                                    hv = sl3[:, :, 8:72].bitcast(FP8) \
                                        .rearrange("p l (h c) -> p l h c", h=nh)
                                else:
                                    hv = sl3[:, :, 2:66] \
                                        .rearrange("p l (h c) -> p l h c", h=nh)
                                wv = wall_t[:, wsec * nh:(wsec + Lgv) * nh] \
                                    .rearrange("p (l h c) -> p l h c",
                                               l=Lgv, h=nh, c=1) \
                                    .to_broadcast([128, Lgv, nh, nch])
                                mv = msg_t[:, wsec * nh * nch:
                                           (wsec + Lgv) * nh * nch] \
                                    .rearrange("p (l h c) -> p l h c",
                                               l=Lgv, h=nh)
                                nc.vector.tensor_tensor(out=mv, in0=hv,
                                                        in1=wv, op=OP.mult)
                                wsec += Lgv
                                woff[g] += Lgv
                            if layer == 1:
                                den = smallp.tile([128, HEADS], F32, tag="den")
                                nc.vector.tensor_reduce(
                                    out=den[:, 0:nh],
                                    in_=wall.rearrange("p (l h) -> p h l",
                                                       l=Lwv),
                                    axis=AX.X, op=OP.add)
                            opre = smallp.tile([128, HEADS * HID], BF16,
                                               tag="opre")
                            nc.vector.tensor_reduce(
                                out=opre[:, 0:nh * nch],
                                in_=msg.rearrange("p (l h c) -> p h c l",
                                                  l=Lwv, h=nh),
                                axis=AX.X, op=OP.add)
                        else:
                            den = smallp.tile([128, HEADS], F32, tag="den")
                            opre = smallp.tile([128, HEADS * HID], BF16,
                                               tag="opre")
                            nc.vector.memset(den[:, 0:nh], 0.0)
                            nc.vector.memset(opre[:, 0:nh * nch], 0.0)
                        nc.vector.tensor_scalar_max(den[:, 0:nh],
                                                    den[:, 0:nh], 1e-30)
                        rec = smallp.tile([128, HEADS], F32, tag="rec")
                        nc.vector.reciprocal(rec[:, 0:nh], den[:, 0:nh])
                        if layer == 1:
                            o1 = smallp.tile([128, 128], BF16, tag="o1")
                            nc.vector.tensor_tensor(
                                out=o1[:].rearrange("p (h c) -> p h c", h=nh),
                                in0=opre[:].rearrange("p (h c) -> p h c",
                                                      h=nh),
                                in1=rec[:].rearrange("p (h c) -> p h c", c=1)
                                    .to_broadcast([128, nh, nch]),
                                op=OP.mult)
                            nc.vector.tensor_tensor(out=o1[:], in0=o1[:],
                                                    in1=b1_t[:], op=OP.add)
                            tneg = smallp.tile([128, 128], BF16, tag="tneg")
                            nc.vector.tensor_scalar_min(tneg[:], o1[:], 0.0)
                            nc.scalar.activation(tneg[:], tneg[:], ACT.Exp,
                                                 0.0, 1.0)
                            nc.vector.tensor_relu(o1[:], o1[:])
                            nc.vector.tensor_tensor(out=o1[:], in0=o1[:],
                                                    in1=tneg[:], op=OP.add)
                            nc.vector.tensor_scalar_add(o1[:], o1[:], -1.0)
                            pst = psump.tile([128, 128], BF16, tag="pst")
                            nc.tensor.transpose(out=pst[:], in_=o1[:],
                                                identity=ident[:])
                            o1T = smallp.tile([128, 128], BF16, tag="o1T")
                            nc.scalar.activation(o1T[:], pst[:], ACT.Copy)
                            ps2 = psump.tile([128, 66], F32, tag="ps2")
                            nc.tensor.matmul(out=ps2[:], lhsT=o1T[:],
                                             rhs=w2r_t[:],
                                             start=True, stop=True)
                            row2 = smallp.tile([128, ROW2], BF16, tag="row2")
                            nc.scalar.activation(row2[:, 0:2].bitcast(F32),
                                                 ps2[:, 64:65], ACT.Copy)
                            nc.scalar.activation(row2[:, 2:66],
                                                 ps2[:, 0:64], ACT.Copy)
                            if SIMINIT:
                                nc.vector.memset(row2[:, 66:128], 0.0)
                            nc.vector.tensor_copy(out=ad2win[:, w:w + 1],
                                                  in_=ps2[:, 65:66])
                            nc.sync.dma_start(
                                out=shard2[w * 128:(w + 1) * 128, :],
                                in_=row2[:])
                        else:
                            o1v = shbuf[:, w * OUT:(w + 1) * OUT]
                            nc.vector.tensor_scalar(
                                out=o1v, in0=opre[:, 0:OUT],
                                scalar1=rec[:, 0:1], scalar2=None,
                                op0=OP.mult)
                            nc.vector.tensor_tensor(out=o1v, in0=o1v,
                                                    in1=b2_t[:], op=OP.add)
                            ex = smallp.tile([128, OUT], F32, tag="ex")
                            nc.scalar.activation(
                                ex[:], o1v, ACT.Exp, 0.0, 1.0,
                                accum_out=sebuf[:, w:w + 1])

                pend = None
                for ws in batches:
                    par = bpar[0] % 2
                    bpar[0] += 1
                    gbase = np.concatenate(
                        [[0], np.cumsum([int(Lg[ws, g].sum())
                                         for g in range(NGRP)])])
                    stag = stagp.tile([128, MAXC * ROW1], BF16, tag="st")
                    gaths = []
                    for g in range(NGRP):
                        cols = int(Lg[ws, g].sum())
                        if cols == 0:
                            continue
                        nidx = 128 * cols
                        ixt = idxp.tile([128, MAXC * 8], I16, tag="ix")
                        nc.sync.dma_start(
                            out=ixt[:, 0:nidx // 16],
                            in_=idxin[:, idx_off:idx_off + nidx // 16])
                        idx_off += nidx // 16
                        sl = stag[:, int(gbase[g]) * ROW1:
                                  (int(gbase[g]) + cols) * ROW1]
                        sl3 = sl.rearrange("p (k d) -> p k d", d=ROW1)
                        gaths.append((g, sl3, ixt, nidx))
                    with tc.tile_critical():
                        for g, sl3, ixt, nidx in gaths:
                            nc.gpsimd.dma_gather(
                                out_ap=sl3, in_ap=tabl[g * NP:(g + 1) * NP, :],
                                idxs_ap=ixt[:, 0:nidx // 16],
                                num_idxs=nidx, num_idxs_reg=nidx,
                                elem_size=ROW1, single_packet=False,
                                prepare_only=True, sem=dma_sems[par],
                            ).then_inc(prep_sem, 1)
                            pcnt[0] += 1
                            nc.gpsimd.wait_ge(prep_sem, pcnt[0])
                            nc.gpsimd.trigger_dma(count=1)
                            gcnt[par] += 1
                        if pend is not None:
                            nc.gpsimd.wait_ge(dma_sems[pend["par"]],
                                              16 * pend["gc"])
                            for sl3p in pend["slices"]:
                                nc.gpsimd.tensor_copy(out=sl3p[:, :, 0:1],
                                                      in_=sl3p[:, :, 0:1])
                            for ixtp in pend["ixts"]:
                                nc.gpsimd.tensor_copy(out=ixtp[0:1, 0:1],
                                                      in_=ixtp[0:1, 0:1])
                    if pend is not None:
                        compute(pend)
                    pend = dict(ws=ws, gbase=gbase, stag=stag,
                                slices=[t[1] for t in gaths],
                                ixts=[t[2] for t in gaths],
                                gc=gcnt[par], par=par)
                with tc.tile_critical():
                    nc.gpsimd.wait_ge(dma_sems[pend["par"]], 16 * pend["gc"])
                    for sl3p in pend["slices"]:
                        nc.gpsimd.tensor_copy(out=sl3p[:, :, 0:1],
                                              in_=sl3p[:, :, 0:1])
                    for ixtp in pend["ixts"]:
                        nc.gpsimd.tensor_copy(out=ixtp[0:1, 0:1],
                                              in_=ixtp[0:1, 0:1])
                compute(pend)

            edge_layer(1)

            # table2 pad row: a_s := -300
            pr2 = constp.tile([1, 2], BF16, tag="pr2")
            nc.vector.memset(pr2[0:1, 0:2].bitcast(F32), A_S_NEG)
            nc.sync.dma_start(out=shard2[PAD2:PAD2 + 1, 0:2], in_=pr2[0:1, :])

            with tc.tile_critical():
                nc.gpsimd.collective_compute(
                    "AllGather", OP.bypass,
                    replica_groups=[list(range(NCORES))],
                    ins=[shard2[:]], outs=[table2[:]],
                ).then_inc(cc_sem, 1)
                nc.gpsimd.wait_ge(cc_sem, 1)

            edge_layer(2)

            # deferred log-softmax normalizer + output
            lnse = constp.tile([128, NW], F32, tag="lnse")
            nc.scalar.activation(lnse[:], sebuf[:], ACT.Ln, 0.0, 1.0)
            for w in range(NW):
                fo = smallp.tile([128, OUT], F32, tag="fo")
                nc.vector.tensor_scalar(
                    out=fo[:], in0=shbuf[:, w * OUT:(w + 1) * OUT],
                    scalar1=lnse[:, w:w + 1], scalar2=None, op0=OP.subtract)
                nc.sync.dma_start(out=outp[w * 128:(w + 1) * 128, :],
                                  in_=fo[:])
    nc.compile()
    return nc


_CACHE = {}


def _batches(Lw):
    batches = []
    cur, acc = [], 0
    for w in range(NW):
        lw = int(Lw[w])
        if cur and (acc + lw > COLS_BUDGET or len(cur) >= 16):
            batches.append(cur)
            cur, acc = [], 0
        cur.append(w)
        acc += lw
    if cur:
        batches.append(cur)
    return batches


def kernel(**inputs):
    ei = np.asarray(inputs["edge_index"])
    src, dst = ei[0].astype(np.int64), ei[1].astype(np.int64)
    lay = _layout(src, dst)
    batches = _batches(lay["Lw"])
    per_core = _host_inputs(inputs, lay, batches)
    key = (ei.tobytes()[:64], int(lay["Lg"].sum()))
    if key not in _CACHE:
        _CACHE[key] = _build_program(lay["Lg"], lay["Lw"], batches)
    nc = _CACHE[key]
    res = run_bass_kernel_spmd(nc, per_core, core_ids=list(range(NCORES)))
    out = np.empty((N, OUT), np.float32)
    for k in range(NCORES):
        out[k * NSH:(k + 1) * NSH] = res.results[k]["out"][lay["poss"][k]]
    return out


if __name__ == "__main__":
    d = np.load("/root/problem/inputs.npz")
    o = kernel(**{k: d[k] for k in d.files})
    ref = np.load("/root/problem/ref_out.npy")
    rel = np.linalg.norm(o - ref) / np.linalg.norm(ref)
    err = np.abs(o - ref) / (np.abs(ref) + 1e-5)
    print("fro rel err:", rel, "max elem rel err:", err.max())


# revision 25
# speedup vs baseline: 1.4211x; 1.4034x over previous
"""GAT (2-layer) Trainium2 Bass kernel — 8-core SPMD.

Sharding: dst nodes across 8 cores (12500 each). Per core, dsts are packed
into 98 windows of 128 (one SBUF partition per dst) by a budget-aware
cross-core packer that minimizes per-(window, src-group) slot columns.
Edge rows are fetched with pipelined gpsimd.dma_gather calls from per-layer
node tables (4 src-groups of 25088 rows to fit int16 indices); attention +
weighted segment-sum run as per-partition DVE ops with lrelu/exp on the
scalar engine. Layer-1 table rows are 256B [a_s bf16 x8 | h fp8 x128];
layer-2 rows are 256B [a_s f32 | h bf16 x64]. Pad slots point at a row with
a_s=-300 => weights ~e^-56, no masking needed. Layer-2 node table is built
per-shard and AllGathered. Layer-2 log-softmax defers the Ln to one batched
pass to avoid activation-table thrash.
"""

import os

import numpy as np
import ml_dtypes

import concourse.bacc as bacc
import concourse.bass as bass
import concourse.mybir as mybir
import concourse.tile as tile
from concourse.bass_utils import run_bass_kernel_spmd
from concourse.masks import make_identity

F32 = mybir.dt.float32
BF16 = mybir.dt.bfloat16
FP8 = mybir.dt.float8e4
I16 = mybir.dt.int16
AX = mybir.AxisListType
OP = mybir.AluOpType
ACT = mybir.ActivationFunctionType

N, E = 100000, 1600000
IN, HID, OUT, HEADS = 256, 16, 64, 8
NEG = 0.2
NCORES = 8
NSH = N // NCORES        # 12500
NGRP = 4
GSZ = N // NGRP          # 25000
NP = 25088               # padded rows per group (196*128)
NW = 98                  # windows per core
SH_ROWS = NW * 128       # 12544
PAD1 = GSZ               # group-local pad row, table1 (25000; rows 25000..25087 zero-x)
PAD2 = NSH               # group-local pad row, table2 (12500 in core 2g's shard)
COLS_BUDGET = 160        # slot columns per gather batch
ROW1 = 128               # bf16 elems per table1 row (256B: a_s 8xbf16 | h 128xfp8 | pad)
ROW2 = 128               # bf16 elems per table2 row (256B: a_s f32 | h2 64xbf16 | pad)
A_S_NEG = -300.0
NCH = 4                  # AllGather chunks (by shard2 row ranges)
WPC = 25                 # windows per chunk (last chunk: 23)
CSZ = [3200, 3200, 3200, 2944]
COFF = [0, 3200, 6400, 9600]


# ---------------------------------------------------------------- host side
def _layout(src, dst):
    core = dst // NSH
    grp = src // GSZ
    cg_all = np.zeros((NCORES, NSH, NGRP), np.int64)
    np.add.at(cg_all, (core, dst % NSH, grp), 1)

    # seed: quantized-profile sort => similar group-profiles share a window
    asgs, cms = [], []
    for k in range(NCORES):
        cg = cg_all[k]
        perm = np.lexsort((cg[:, 3], cg[:, 2] // 3, cg[:, 1] // 4,
                           cg[:, 0] // 5))[::-1]
        asg = np.empty(NSH, np.int64)
        asg[perm] = np.arange(NSH) // 128
        cm = np.zeros((NW, NGRP), np.int64)
        np.maximum.at(cm, asg, cg)
        asgs.append(asg)
        cms.append(cm)
    # refinement: repack each core into the budgets implied by the others
    BIG = 1 << 60
    for _ in range(2):
        for k in range(NCORES):
            others = np.maximum.reduce(
                [cms[j] for j in range(NCORES) if j != k])
            cg = cg_all[k]
            order = np.argsort(-cg.sum(1), kind="stable")
            used = np.zeros(NW, np.int64)
            cm = np.zeros((NW, NGRP), np.int64)
            asg = np.empty(NSH, np.int64)
            eff = others.copy()
            for d in order:
                c = cg[d]
                delta = np.maximum(0, c[None, :] - eff).sum(1)
                delta[used >= 128] = BIG
                cand = np.flatnonzero(delta == delta.min())
                if len(cand) > 1:
                    slack = (eff[cand]
                             - np.minimum(c[None, :], eff[cand])).sum(1)
                    w = cand[np.argmin(slack)]
                else:
                    w = cand[0]
                asg[d] = w
                used[w] += 1
                nm = np.maximum(cm[w], c)
                if (nm != cm[w]).any():
                    cm[w] = nm
                    eff[w] = np.maximum(others[w], cm[w])
            asgs[k], cms[k] = asg, cm
    Lg = np.maximum.reduce(cms)
    Lw = Lg.sum(axis=1)

    sig = np.empty(N, np.int64)
    poss = []
    for k in range(NCORES):
        asg = asgs[k]
        o = np.lexsort((np.arange(NSH), asg))
        counts = np.bincount(asg, minlength=NW)
        starts = np.concatenate([[0], np.cumsum(counts)[:-1]])
        r = np.arange(NSH) - np.repeat(starts, counts)
        pos = np.empty(NSH, np.int64)
        pos[o] = asg[o] * 128 + r
        poss.append(pos)
        sig[k * NSH:(k + 1) * NSH] = k * SH_ROWS + pos

    eorder = np.lexsort((grp, dst))
    es, ed, eg, ec = src[eorder], dst[eorder], grp[eorder], core[eorder]
    core_starts = np.searchsorted(ec, np.arange(NCORES + 1))
    cores = [(es[a:b], (ed[a:b] - k * NSH), eg[a:b])
             for k, (a, b) in enumerate(zip(core_starts[:-1], core_starts[1:]))]
    return dict(Lg=Lg, Lw=Lw, poss=poss, sig=sig, cores=cores)


def _pack_idx(arr_pj):
    """[128, cols] slot-array of indices -> wrapped idx tile [128, cols*8]."""
    I = arr_pj.T.ravel()                      # I[j*128+p]
    W = I.reshape(-1, 16).T.astype(np.int16)  # [16, len/16]
    return np.tile(W, (8, 1))


def _host_inputs(inputs, lay, batches):
    x = np.asarray(inputs["x"], np.float32)
    W1 = np.asarray(inputs["W1"], np.float64)
    att1_s = np.asarray(inputs["att1_s"], np.float64)
    att1_d = np.asarray(inputs["att1_d"], np.float64)
    W2 = np.asarray(inputs["W2"], np.float64)
    att2_s = np.asarray(inputs["att2_s"], np.float64)
    att2_d = np.asarray(inputs["att2_d"], np.float64)
    b1 = np.asarray(inputs["b1"], np.float32)
    b2 = np.asarray(inputs["b2"], np.float32)
    Lg, poss, sig = lay["Lg"], lay["poss"], lay["sig"]

    A_s = np.zeros((HEADS * HID, HEADS))
    A_d = np.zeros((HEADS * HID, HEADS))
    for h in range(HEADS):
        A_s[h * HID:(h + 1) * HID, h] = att1_s[h]
        A_d[h * HID:(h + 1) * HID, h] = att1_d[h]
    w1r = np.concatenate([W1, W1 @ A_s, W1 @ A_d], axis=1)          # [256,144]
    w2r = np.concatenate([W2, W2 @ att2_s.T, W2 @ att2_d.T], axis=1)  # [128,66]
    w1r_bf = w1r.astype(ml_dtypes.bfloat16)
    w2r_bf = w2r.astype(ml_dtypes.bfloat16)

    xT = np.zeros((IN, NGRP * NP), np.float32)
    for g in range(NGRP):
        xT[:, g * NP:g * NP + GSZ] = x[g * GSZ:(g + 1) * GSZ].T
    xT_bf = xT.astype(ml_dtypes.bfloat16)

    common = {
        "xt0": np.ascontiguousarray(xT_bf[:128]),
        "xt1": np.ascontiguousarray(xT_bf[128:]),
        "w1r0": np.ascontiguousarray(w1r_bf[:128]),
        "w1r1": np.ascontiguousarray(w1r_bf[128:]),
        "w2r": np.ascontiguousarray(w2r_bf),
        "b1rep": np.ascontiguousarray(
            np.tile(b1[None, :], (128, 1)).astype(ml_dtypes.bfloat16)),
        "b2rep": np.ascontiguousarray(
            np.tile(b2[None, :], (128, 1)).astype(np.float32)),
    }

    per_core = []
    for k in range(NCORES):
        es, edl, eg = lay["cores"][k]
        pos = poss[k]
        o = np.lexsort((eg, pos[edl]))
        es_o, eg_o, pos_o = es[o], eg[o], pos[edl][o]
        w_o, p_o = pos_o // 128, pos_o % 128
        key = pos_o * NGRP + eg_o
        slot = np.arange(len(o)) - np.searchsorted(key, key)
        idx1_secs, idx2_secs = [], []
        for ws in batches:
            for g in range(NGRP):
                cols = int(Lg[ws, g].sum())
                if cols == 0:
                    continue
                a1 = np.full((128, cols), PAD1, np.int64)
                a2 = np.full((128, cols), PAD2, np.int64)
                coff = 0
                for w in ws:
                    m = (w_o == w) & (eg_o == g)
                    pp, jj, ss = p_o[m], slot[m], es_o[m]
                    a1[pp, coff + jj] = ss % GSZ
                    a2[pp, coff + jj] = sig[ss] % NP
                    coff += int(Lg[w, g])
                idx1_secs.append(a1)
                idx2_secs.append(a2)
        idx1 = np.concatenate([_pack_idx(a) for a in idx1_secs], axis=1)
        idx2 = np.concatenate([_pack_idx(a) for a in idx2_secs], axis=1)
        xtp = np.zeros((IN, SH_ROWS), np.float32)
        xtp[:, pos] = x[k * NSH:(k + 1) * NSH].T
        xtp_bf = xtp.astype(ml_dtypes.bfloat16)
        d = dict(common)
        d["idx1"] = np.ascontiguousarray(idx1)
        d["idx2"] = np.ascontiguousarray(idx2)
        d["xtp0"] = np.ascontiguousarray(xtp_bf[:128])
        d["xtp1"] = np.ascontiguousarray(xtp_bf[128:])
        per_core.append(d)
    return per_core


# ------------------------------------------------------------- device side
def _build_program(Lg, Lw, batches):
    nc = bacc.Bacc("TRN2", target_bir_lowering=False, debug=False,
                   num_devices=NCORES)
    IDXF = int(Lg.sum()) * 8
    LWMAX = int(Lw.max())
    MAXC = max(COLS_BUDGET, LWMAX)
    SIMINIT = int(os.environ.get("GAT_SIMINIT", "0"))
    xt0 = nc.declare_dram_parameter("xt0", [128, NGRP * NP], BF16, isOutput=False)
    xt1 = nc.declare_dram_parameter("xt1", [128, NGRP * NP], BF16, isOutput=False)
    w1r0 = nc.declare_dram_parameter("w1r0", [128, 144], BF16, isOutput=False)
    w1r1 = nc.declare_dram_parameter("w1r1", [128, 144], BF16, isOutput=False)
    w2r = nc.declare_dram_parameter("w2r", [128, 66], BF16, isOutput=False)
    b1rep = nc.declare_dram_parameter("b1rep", [128, 128], BF16, isOutput=False)
    b2rep = nc.declare_dram_parameter("b2rep", [128, 64], F32, isOutput=False)
    idx1 = nc.declare_dram_parameter("idx1", [128, IDXF], I16, isOutput=False)
    idx2 = nc.declare_dram_parameter("idx2", [128, IDXF], I16, isOutput=False)
    xtp0 = nc.declare_dram_parameter("xtp0", [128, SH_ROWS], BF16, isOutput=False)
    xtp1 = nc.declare_dram_parameter("xtp1", [128, SH_ROWS], BF16, isOutput=False)
    outp = nc.declare_dram_parameter("out", [SH_ROWS, OUT], F32, isOutput=True)

    table1 = nc.dram_tensor("table1", [NGRP * NP, ROW1], BF16)
    shard2 = nc.dram_tensor("shard2", [SH_ROWS, ROW2], BF16)
    table2 = nc.dram_tensor("table2", [NCORES * SH_ROWS, ROW2], BF16,
                            addr_space="Shared")

    dma_sems = [nc.alloc_semaphore("g_dmaA"), nc.alloc_semaphore("g_dmaB")]
    prep_sem = nc.alloc_semaphore("g_prep")
    cc_sem = nc.alloc_semaphore("cc")
    gcnt = [0, 0]
    pcnt = [0]
    bpar = [0]

    TPB, BLK = 196, 14

    with tile.TileContext(nc) as tc:
        with (
            tc.tile_pool(name="const", bufs=1) as constp,
            tc.tile_pool(name="psum", bufs=2, space="PSUM") as psump,
            tc.tile_pool(name="stag", bufs=2) as stagp,
            tc.tile_pool(name="idx", bufs=8) as idxp,
            tc.tile_pool(name="work", bufs=2) as workp,
            tc.tile_pool(name="small", bufs=3) as smallp,
        ):
            w1r0_t = constp.tile([128, 144], BF16, tag="w1r0")
            w1r1_t = constp.tile([128, 144], BF16, tag="w1r1")
            w2r_t = constp.tile([128, 66], BF16, tag="w2r")
            b1_t = constp.tile([128, 128], BF16, tag="b1")
            b2_t = constp.tile([128, 64], F32, tag="b2")
            ident = constp.tile([128, 128], BF16, tag="ident")
            adwin = constp.tile([128, NW * HEADS], BF16, tag="adwin")
            ad2win = constp.tile([128, NW], F32, tag="ad2win")
            shbuf = constp.tile([128, NW * OUT], F32, tag="shbuf")
            sebuf = constp.tile([128, NW], F32, tag="sebuf")
            nc.sync.dma_start(out=w1r0_t[:], in_=w1r0[:])
            nc.sync.dma_start(out=w1r1_t[:], in_=w1r1[:])
            nc.sync.dma_start(out=w2r_t[:], in_=w2r[:])
            nc.sync.dma_start(out=b1_t[:], in_=b1rep[:])
            nc.sync.dma_start(out=b2_t[:], in_=b2rep[:])
            make_identity(nc, ident[:])

            # ---------------- phase 0: dense h1 table (all nodes) ----------
            with (
                tc.tile_pool(name="xt", bufs=2) as xtpool,
                tc.tile_pool(name="dense", bufs=2) as densep,
            ):
                for g in range(NGRP):
                    for blk in range(TPB // BLK):
                        base = g * NP + blk * BLK * 128
                        xs0 = xtpool.tile([128, BLK * 128], BF16, tag="xs0")
                        xs1 = xtpool.tile([128, BLK * 128], BF16, tag="xs1")
                        nc.sync.dma_start(out=xs0[:],
                                          in_=xt0[:, base:base + BLK * 128])
                        nc.sync.dma_start(out=xs1[:],
                                          in_=xt1[:, base:base + BLK * 128])
                        rows = densep.tile([128, BLK * ROW1], BF16, tag="rows")
                        for t in range(BLK):
                            ps = psump.tile([128, 144], F32, tag="ps0")
                            nc.tensor.matmul(
                                out=ps[:], lhsT=xs0[:, t * 128:(t + 1) * 128],
                                rhs=w1r0_t[:], start=True, stop=False)
                            nc.tensor.matmul(
                                out=ps[:], lhsT=xs1[:, t * 128:(t + 1) * 128],
                                rhs=w1r1_t[:], start=False, stop=True)
                            rv = rows[:, t * ROW1:(t + 1) * ROW1]
                            nc.scalar.activation(rv[:, 0:8], ps[:, 128:136],
                                                 ACT.Copy)
                            nc.scalar.activation(rv[:, 8:72].bitcast(FP8),
                                                 ps[:, 0:128], ACT.Copy)
                            if SIMINIT:
                                nc.vector.memset(rv[:, 72:128], 0.0)
                        nc.sync.dma_start(
                            out=table1[base:base + BLK * 128, :]
                                .rearrange("(a p) r -> p a r", p=128),
                            in_=rows[:].rearrange("p (a r) -> p a r", a=BLK))
                # pad row: a_s := -300 (h stays 0) on group-local row PAD1
                padrow = constp.tile([128, ROW1], BF16, tag="padrow")
                nc.vector.memset(padrow[:], 0.0)
                nc.vector.memset(padrow[0:1, 0:8], A_S_NEG)
                for g in range(NGRP):
                    nc.sync.dma_start(
                        out=table1[g * NP + PAD1:g * NP + PAD1 + 1, :],
                        in_=padrow[0:1, :])

                # a_d per window (window-ordered x.T)
                for w in range(NW):
                    xp0 = xtpool.tile([128, 128], BF16, tag="xp0")
                    xp1 = xtpool.tile([128, 128], BF16, tag="xp1")
                    nc.sync.dma_start(out=xp0[:],
                                      in_=xtp0[:, w * 128:(w + 1) * 128])
                    nc.sync.dma_start(out=xp1[:],
                                      in_=xtp1[:, w * 128:(w + 1) * 128])
                    psa = psump.tile([128, 16], F32, tag="psa")
                    nc.tensor.matmul(out=psa[:], lhsT=xp0[:],
                                     rhs=w1r0_t[:, 128:144],
                                     start=True, stop=False)
                    nc.tensor.matmul(out=psa[:], lhsT=xp1[:],
                                     rhs=w1r1_t[:, 128:144],
                                     start=False, stop=True)
                    nc.scalar.activation(adwin[:, w * 8:(w + 1) * 8],
                                         psa[:, 8:16], ACT.Copy)

            # ---------------- edge layers ----------------------------------
            def edge_layer(layer):
                if layer == 1:
                    idxin, nh, nch = idx1, HEADS, HID
                    Lg_, Lw_, batches_ = Lg, Lw, batches
                    tab = lambda g: table1[g * NP:(g + 1) * NP, :]
                else:
                    idxin, nh, nch = idx2, 1, OUT
                    Lg_, Lw_, batches_ = Lg, Lw, batches
                    tab = lambda g: table2[g * NP:(g + 1) * NP, :]
                idx_off = 0

                def compute(pd):
                    ws, gbase, stag = pd["ws"], pd["gbase"], pd["stag"]
                    woff = np.zeros(NGRP, np.int64)
                    for w in ws:
                        Lwv = int(Lw_[w])
                        wall_t = workp.tile([128, LWMAX * HEADS], BF16, tag="wa")
                        wall = wall_t[:, 0:Lwv * nh]
                        msg_t = workp.tile([128, LWMAX * HEADS * HID], BF16,
                                           tag="mg")
                        msg = msg_t[:, 0:Lwv * nh * nch]
                        if Lwv > 0:
                            wsec = 0
                            for g in range(NGRP):
                                Lgv = int(Lg_[w, g])
                                if Lgv == 0:
                                    continue
                                c0 = int(gbase[g] + woff[g])
                                sl3 = stag[:, c0 * ROW1:(c0 + Lgv) * ROW1] \
                                    .rearrange("p (l r) -> p l r", l=Lgv)
                                if layer == 1:
                                    a_s = sl3[:, :, 0:8]
                                    adv = adwin[:, w * 8:(w + 1) * 8]
                                else:
                                    a_s = sl3[:, :, 0:2].bitcast(F32)
                                    adv = ad2win[:, w:w + 1]
                                adv = adv.rearrange("p (l h) -> p l h", l=1) \
                                    .to_broadcast([128, Lgv, nh])
                                uv = wall_t[:, wsec * nh:(wsec + Lgv) * nh] \
                                    .rearrange("p (l h) -> p l h", l=Lgv)
                                nc.vector.tensor_tensor(out=uv, in0=a_s,
                                                        in1=adv, op=OP.add)
                                wsec += Lgv
                            lr = workp.tile([128, LWMAX * HEADS], BF16,
                                            tag="lr")
                            nc.vector.tensor_scalar_mul(lr[:, 0:Lwv * nh],
                                                        wall, NEG)
                            nc.vector.tensor_tensor(out=wall, in0=wall,
                                                    in1=lr[:, 0:Lwv * nh],
                                                    op=OP.max)
                            if layer == 2:
                                den = smallp.tile([128, HEADS], F32, tag="den")
                                nc.scalar.activation(
                                    wall, wall, ACT.Exp, 0.0, 1.0,
                                    accum_out=den[:, 0:1])
                            else:
                                nc.scalar.activation(wall, wall, ACT.Exp,
                                                     0.0, 1.0)
                            # weighted messages
                            wsec = 0
                            for g in range(NGRP):
                                Lgv = int(Lg_[w, g])
                                if Lgv == 0:
                                    continue
                                c0 = int(gbase[g] + woff[g])
                                sl3 = stag[:, c0 * ROW1:(c0 + Lgv) * ROW1] \
                                    .rearrange("p (l r) -> p l r", l=Lgv)
                                if layer == 1:
                                    hv = sl3[:, :, 8:72].bitcast(FP8) \
                                        .rearrange("p l (h c) -> p l h c", h=nh)
                                else:
                                    hv = sl3[:, :, 2:66] \
                                        .rearrange("p l (h c) -> p l h c", h=nh)
                                wv = wall_t[:, wsec * nh:(wsec + Lgv) * nh] \
                                    .rearrange("p (l h c) -> p l h c",
                                               l=Lgv, h=nh, c=1) \
                                    .to_broadcast([128, Lgv, nh, nch])
                                mv = msg_t[:, wsec * nh * nch:
                                           (wsec + Lgv) * nh * nch] \
                                    .rearrange("p (l h c) -> p l h c",
                                               l=Lgv, h=nh)
                                nc.vector.tensor_tensor(out=mv, in0=hv,
                                                        in1=wv, op=OP.mult)
                                wsec += Lgv
                                woff[g] += Lgv
                            if layer == 1:
                                den = smallp.tile([128, HEADS], F32, tag="den")
                                nc.vector.tensor_reduce(
                                    out=den[:, 0:nh],
                                    in_=wall.rearrange("p (l h) -> p h l",
                                                       l=Lwv),
                                    axis=AX.X, op=OP.add)
                            opre = smallp.tile([128, HEADS * HID], F32,
                                               tag="opre")
                            nc.vector.tensor_reduce(
                                out=opre[:, 0:nh * nch],
                                in_=msg.rearrange("p (l h c) -> p h c l",
                                                  l=Lwv, h=nh),
                                axis=AX.X, op=OP.add)
                        else:
                            den = smallp.tile([128, HEADS], F32, tag="den")
                            opre = smallp.tile([128, HEADS * HID], F32,
                                               tag="opre")
                            nc.vector.memset(den[:, 0:nh], 0.0)
                            nc.vector.memset(opre[:, 0:nh * nch], 0.0)
                        nc.vector.tensor_scalar_max(den[:, 0:nh],
                                                    den[:, 0:nh], 1e-30)
                        rec = smallp.tile([128, HEADS], F32, tag="rec")
                        nc.vector.reciprocal(rec[:, 0:nh], den[:, 0:nh])
                        if layer == 1:
                            o1 = smallp.tile([128, 128], BF16, tag="o1")
                            nc.vector.tensor_tensor(
                                out=o1[:].rearrange("p (h c) -> p h c", h=nh),
                                in0=opre[:].rearrange("p (h c) -> p h c",
                                                      h=nh),
                                in1=rec[:].rearrange("p (h c) -> p h c", c=1)
                                    .to_broadcast([128, nh, nch]),
                                op=OP.mult)
                            nc.vector.tensor_tensor(out=o1[:], in0=o1[:],
                                                    in1=b1_t[:], op=OP.add)
                            tneg = smallp.tile([128, 128], BF16, tag="tneg")
                            nc.vector.tensor_scalar_min(tneg[:], o1[:], 0.0)
                            nc.scalar.activation(tneg[:], tneg[:], ACT.Exp,
                                                 0.0, 1.0)
                            nc.vector.tensor_relu(o1[:], o1[:])
                            nc.vector.tensor_tensor(out=o1[:], in0=o1[:],
                                                    in1=tneg[:], op=OP.add)
                            nc.vector.tensor_scalar_add(o1[:], o1[:], -1.0)
                            pst = psump.tile([128, 128], BF16, tag="pst")
                            nc.tensor.transpose(out=pst[:], in_=o1[:],
                                                identity=ident[:])
                            o1T = smallp.tile([128, 128], BF16, tag="o1T")
                            nc.scalar.activation(o1T[:], pst[:], ACT.Copy)
                            ps2 = psump.tile([128, 66], F32, tag="ps2")
                            nc.tensor.matmul(out=ps2[:], lhsT=o1T[:],
                                             rhs=w2r_t[:],
                                             start=True, stop=True)
                            row2 = smallp.tile([128, ROW2], BF16, tag="row2")
                            nc.scalar.activation(row2[:, 0:2].bitcast(F32),
                                                 ps2[:, 64:65], ACT.Copy)
                            nc.scalar.activation(row2[:, 2:66],
                                                 ps2[:, 0:64], ACT.Copy)
                            if SIMINIT:
                                nc.vector.memset(row2[:, 66:128], 0.0)
                            nc.vector.tensor_copy(out=ad2win[:, w:w + 1],
                                                  in_=ps2[:, 65:66])
                            nc.sync.dma_start(
                                out=shard2[w * 128:(w + 1) * 128, :],
                                in_=row2[:])
                        else:
                            o1v = shbuf[:, w * OUT:(w + 1) * OUT]
                            nc.vector.tensor_scalar(
                                out=o1v, in0=opre[:, 0:OUT],
                                scalar1=rec[:, 0:1], scalar2=None,
                                op0=OP.mult)
                            nc.vector.tensor_tensor(out=o1v, in0=o1v,
                                                    in1=b2_t[:], op=OP.add)
                            ex = smallp.tile([128, OUT], F32, tag="ex")
                            nc.scalar.activation(
                                ex[:], o1v, ACT.Exp, 0.0, 1.0,
                                accum_out=sebuf[:, w:w + 1])

                pend = None
                for ws in batches_:
                    par = bpar[0] % 2
                    bpar[0] += 1
                    gbase = np.concatenate(
                        [[0], np.cumsum([int(Lg_[ws, g].sum())
                                         for g in range(NGRP)])])
                    stag = stagp.tile([128, MAXC * ROW1], BF16, tag="st")
                    gaths = []
                    for g in range(NGRP):
                        cols = int(Lg_[ws, g].sum())
                        if cols == 0:
                            continue
                        nidx = 128 * cols
                        ixt = idxp.tile([128, MAXC * 8], I16, tag="ix")
                        nc.sync.dma_start(
                            out=ixt[:, 0:nidx // 16],
                            in_=idxin[:, idx_off:idx_off + nidx // 16])
                        idx_off += nidx // 16
                        sl = stag[:, int(gbase[g]) * ROW1:
                                  (int(gbase[g]) + cols) * ROW1]
                        sl3 = sl.rearrange("p (k d) -> p k d", d=ROW1)
                        gaths.append((g, sl3, ixt, nidx))
                    with tc.tile_critical():
                        for g, sl3, ixt, nidx in gaths:
                            nc.gpsimd.dma_gather(
                                out_ap=sl3, in_ap=tab(g),
                                idxs_ap=ixt[:, 0:nidx // 16],
                                num_idxs=nidx, num_idxs_reg=nidx,
                                elem_size=ROW1, single_packet=False,
                                prepare_only=True, sem=dma_sems[par],
                            ).then_inc(prep_sem, 1)
                            pcnt[0] += 1
                            nc.gpsimd.wait_ge(prep_sem, pcnt[0])
                            nc.gpsimd.trigger_dma(count=1)
                            gcnt[par] += 1
                        if pend is not None:
                            nc.gpsimd.wait_ge(dma_sems[pend["par"]],
                                              16 * pend["gc"])
                            for sl3p in pend["slices"]:
                                nc.gpsimd.tensor_copy(out=sl3p[:, :, 0:1],
                                                      in_=sl3p[:, :, 0:1])
                            for ixtp in pend["ixts"]:
                                nc.gpsimd.tensor_copy(out=ixtp[0:1, 0:1],
                                                      in_=ixtp[0:1, 0:1])
                    if pend is not None:
                        compute(pend)
                    pend = dict(ws=ws, gbase=gbase, stag=stag,
                                slices=[t[1] for t in gaths],
                                ixts=[t[2] for t in gaths],
                                gc=gcnt[par], par=par)
                with tc.tile_critical():
                    nc.gpsimd.wait_ge(dma_sems[pend["par"]], 16 * pend["gc"])
                    for sl3p in pend["slices"]:
                        nc.gpsimd.tensor_copy(out=sl3p[:, :, 0:1],
                                              in_=sl3p[:, :, 0:1])
                    for ixtp in pend["ixts"]:
                        nc.gpsimd.tensor_copy(out=ixtp[0:1, 0:1],
                                              in_=ixtp[0:1, 0:1])
                compute(pend)

            edge_layer(1)

            # table2 pad row: a_s := -300 (after L1 so it is not overwritten)
            pr2 = constp.tile([1, 2], BF16, tag="pr2")
            nc.vector.memset(pr2[0:1, 0:2].bitcast(F32), A_S_NEG)
            nc.sync.dma_start(out=shard2[PAD2:PAD2 + 1, 0:2], in_=pr2[0:1, :])

            with tc.tile_critical():
                nc.gpsimd.collective_compute(
                    "AllGather", OP.bypass,
                    replica_groups=[list(range(NCORES))],
                    ins=[shard2[:]], outs=[table2[:]],
                ).then_inc(cc_sem, 1)
                nc.gpsimd.wait_ge(cc_sem, 1)

            edge_layer(2)

            # deferred log-softmax normalizer + output
            lnse = constp.tile([128, NW], F32, tag="lnse")
            nc.scalar.activation(lnse[:], sebuf[:], ACT.Ln, 0.0, 1.0)
            for w in range(NW):
                fo = smallp.tile([128, OUT], F32, tag="fo")
                nc.vector.tensor_scalar(
                    out=fo[:], in0=shbuf[:, w * OUT:(w + 1) * OUT],
                    scalar1=lnse[:, w:w + 1], scalar2=None, op0=OP.subtract)
                nc.sync.dma_start(out=outp[w * 128:(w + 1) * 128, :],
                                  in_=fo[:])
    nc.compile()
    return nc


_CACHE = {}


def _batches(Lw):
    batches = []
    cur, acc = [], 0
    for w in range(NW):
        lw = int(Lw[w])
        if cur and (acc + lw > COLS_BUDGET or len(cur) >= 16):
            batches.append(cur)
            cur, acc = [], 0
        cur.append(w)
        acc += lw
    if cur:
        batches.append(cur)
    return batches


def kernel(**inputs):
    ei = np.asarray(inputs["edge_index"])
    src, dst = ei[0].astype(np.int64), ei[1].astype(np.int64)
    lay = _layout(src, dst)
    batches = _batches(lay["Lw"])
    per_core = _host_inputs(inputs, lay, batches)
    key = (ei.tobytes()[:64], int(lay["Lg"].sum()))
    if key not in _CACHE:
        _CACHE[key] = _build_program(lay["Lg"], lay["Lw"], batches)
    nc = _CACHE[key]
    res = run_bass_kernel_spmd(nc, per_core, core_ids=list(range(NCORES)))
    out = np.empty((N, OUT), np.float32)
    for k in range(NCORES):
        out[k * NSH:(k + 1) * NSH] = res.results[k]["out"][lay["poss"][k]]
    return out


if __name__ == "__main__":
    d = np.load("/root/problem/inputs.npz")
    o = kernel(**{k: d[k] for k in d.files})
    ref = np.load("/root/problem/ref_out.npy")
    rel = np.linalg.norm(o - ref) / np.linalg.norm(ref)
    err = np.abs(o - ref) / (np.abs(ref) + 1e-5)
    print("fro rel err:", rel, "max elem rel err:", err.max())
